# revision 50
# baseline (speedup 1.0000x reference)
"""Trainium2 Bass kernel for nn_Attention_78280073937702.

Dense transformer attention block (prefill, B=1, S=2048, H=4096, 32 heads,
head_dim=128, fp32) sharded tensor-parallel over heads across 8 NeuronCores
(4 heads per core).

Weights are pre-transposed (and bf16-cast) on the host so every projection
uses them directly as the stationary matmul operand; only the hidden
activations are transposed on-device (PE transpose).  The per-core schedule
software-pipelines the engines so the PE never idles (avoiding the TRN2
p-state down-clock):

  phase 1:  stream hidden -> cast bf16 -> PE-transpose into hidT [h, s];
            V projection (natural [s, d]) interleaved per s-block
  phase 2:  per head h: Q^T/K^T projection (PSUM accum over h-tiles) + RoPE
            (PE half-swap matmul + bf16 DVE ops), with head h-1's attention
            (S^T -> exp -> masked diag -> DVE running-sum -> U^T accum ->
            1/Z -> normalize) emitted interleaved so its ACT/DVE work hides
            under head h's projection matmuls; AllGather per head
  phase 3:  head 3's attention interleaved with o_proj (Wo^T stationary,
            gathered attn^T moving); head-3 gathers split per s-slice so the
            collective latency hides under o_proj of earlier slices.

Host side: shards W_pack/W_o by head, pre-transposes + casts weights to
bf16, builds bf16 RoPE tables, concatenates the 8 per-core out^T slices and
transposes to [1, S, H].
"""

import os
import sys
from contextlib import ExitStack

import numpy as np

for _p in ("/opt/trn_rl_repo", os.path.expanduser("~/.axon_site/_ro/trn_rl_repo")):
    if os.path.isdir(_p) and _p not in sys.path:
        sys.path.insert(0, _p)

import ml_dtypes  # noqa: E402

import concourse.bacc as bacc  # noqa: E402
import concourse.mybir as mybir  # noqa: E402
import concourse.tile as tile  # noqa: E402
from concourse.alu_op_type import AluOpType  # noqa: E402
from concourse.bass_utils import run_bass_kernel_spmd  # noqa: E402

F32 = mybir.dt.float32
F32R = mybir.dt.float32r
BF16 = mybir.dt.bfloat16
EXPF = mybir.ActivationFunctionType.Exp

N_CORES = 8
S = 2048
H = 4096
D = 128
N_HEADS = 32
NH_LOC = N_HEADS // N_CORES  # 4 heads per core
P = 128
HT = H // P  # 32 h-tiles
ST = S // P  # 16 s-tiles
SL = 512  # s-slice width for matmul free dim
NSL = S // SL  # 4
VC = NH_LOC * D  # 512 local v columns
OC = H // N_CORES  # 512 output columns per core
NORM = 1.0 / float(np.sqrt(D))


def build_nc():
    nc = bacc.Bacc("TRN2", target_bir_lowering=False, num_devices=N_CORES)

    hid_d = nc.dram_tensor("hidden", [S, H], F32, kind="ExternalInput")
    wqk_d = nc.dram_tensor("wqk_t", [H, 2 * VC], BF16, kind="ExternalInput")
    wv_d = nc.dram_tensor("wv_t", [H, VC], BF16, kind="ExternalInput")
    wo_d = nc.dram_tensor("wo_t", [H, OC], BF16, kind="ExternalInput")
    cos_d = nc.dram_tensor("cos_t", [D, S], BF16, kind="ExternalInput")
    sin_d = nc.dram_tensor("sin_t", [D, S], BF16, kind="ExternalInput")
    out_d = nc.dram_tensor("out_t", [OC, S], BF16, kind="ExternalOutput")

    with tile.TileContext(nc) as tc, ExitStack() as ctx:
        dram = ctx.enter_context(tc.tile_pool(name="dram", bufs=1, space="DRAM"))
        attn_loc = dram.tile([NH_LOC, NSL, D, SL], BF16)
        # full-head gather buffers for heads 0..2; per-s-slice for head 3
        attn_all = [
            dram.tile([N_CORES, NSL, D, SL], BF16, addr_space="Shared",
                      name=f"attn_all{h}")
            for h in range(NH_LOC - 1)
        ]
        attn_all3 = [
            dram.tile([N_CORES, D, SL], BF16, addr_space="Shared",
                      name=f"attn_all3_{j}")
            for j in range(NSL)
        ]

        # ---------------- constants ----------------
        consts = ctx.enter_context(tc.tile_pool(name="consts", bufs=1))
        identb = consts.tile([P, P], BF16)
        tri01 = consts.tile([P, P], BF16)
        ones_r = consts.tile([P, P], F32R)
        with tc.tile_pool(name="csetup", bufs=1) as csetup:
            ones_t = csetup.tile([P, P], F32)
            nc.gpsimd.memset(ones_t, 1.0)
            ident = csetup.tile([P, P], F32)
            nc.gpsimd.affine_select(
                out=ident, in_=ones_t, compare_op=AluOpType.is_equal,
                fill=0.0, base=0, channel_multiplier=1, pattern=[[-1, P]],
            )
            nc.vector.tensor_copy(identb, ident)
            # upper-triangular-with-diag 0/1 keep-mask in [k, q] layout
            # (keep q >= k)
            tri_f = csetup.tile([P, P], F32)
            nc.gpsimd.affine_select(
                out=tri_f, in_=ones_t, compare_op=AluOpType.is_ge,
                fill=0.0, base=0, channel_multiplier=-1, pattern=[[1, P]],
            )
            nc.vector.tensor_copy(tri01, tri_f)
            nc.vector.tensor_copy(ones_r, ones_t)

        # ---------------- persistent SBUF ----------------
        trig = ctx.enter_context(tc.tile_pool(name="trig", bufs=1))
        cosT = trig.tile([D, S], BF16)
        sinT = trig.tile([D, S], BF16)

        vnat_pool = ctx.enter_context(tc.tile_pool(name="vnat", bufs=1))
        v_nat = vnat_pool.tile([P, ST, VC], BF16)  # V natural [s, d], 16 KB/part

        qkT_tiles = [None] * NH_LOC  # per-head roped q^T/k^T bf16 [P, 2, S]

        with ExitStack() as abH:  # hidT lives through phases 1+2
            hidT_pool = abH.enter_context(tc.tile_pool(name="hidT", bufs=1))
            hidT = hidT_pool.tile([P, HT, S], BF16)  # 128 KB/part

            # ---------- phase 1: hidden^T + V projection ----------
            with ExitStack() as s1:
                nat = s1.enter_context(tc.tile_pool(name="nat", bufs=3))
                natb = s1.enter_context(tc.tile_pool(name="natb", bufs=2))
                tps = s1.enter_context(
                    tc.tile_pool(name="tpsum", bufs=2, space="PSUM"))
                vps_pool = s1.enter_context(
                    tc.tile_pool(name="vpsum", bufs=2, space="PSUM"))
                wv_pool = s1.enter_context(tc.tile_pool(name="wv", bufs=1))
                wv = wv_pool.tile([P, HT, VC], BF16)  # 32 KB/part

                for st in range(ST):
                    if st == 1:
                        nc.sync.dma_start(cosT, cos_d[:, :])
                        nc.sync.dma_start(sinT, sin_d[:, :])
                    # finer chunks for the first block so the PE starts early
                    nch = 8 if st == 0 else 4
                    cw = H // nch  # chunk width
                    for q4 in range(nch):
                        nt = nat.tile([P, H // 4], F32, tag="nt")
                        nc.sync.dma_start(
                            nt[:, :cw],
                            hid_d[st * P:(st + 1) * P, q4 * cw:(q4 + 1) * cw])
                        ntb = natb.tile([P, H // 4], BF16, tag="ntb")
                        nc.vector.tensor_copy(ntb[:, :cw], nt[:, :cw])
                        for g in range(cw // SL):
                            ps = tps.tile([P, 4, P], F32, tag="t")
                            for u4 in range(4):
                                # transpose as a plain matmul (ntb^T @ I) —
                                # normal mode streams at the full PE rate,
                                # transpose mode runs ~1.6x slower here
                                nc.tensor.matmul(
                                    ps[:, u4, :],
                                    ntb[:, (g * 4 + u4) * P:(g * 4 + u4 + 1) * P],
                                    identb, start=True, stop=True)
                            hb = (q4 * cw) // P + g * 4
                            # split the PSUM->SBUF staging between ACT and DVE
                            dst = hidT[:, hb:hb + 4, st * P:(st + 1) * P]
                            if hb % 8 < 4:
                                nc.scalar.copy(dst, ps)
                            else:
                                nc.vector.tensor_copy(dst, ps)
                    if st == 0:
                        # issued after the first hidden block so its chunks
                        # win the DMA queues and the PE starts sooner
                        nc.sync.dma_start(
                            wv, wv_d.rearrange("(t p) c -> p t c", p=P))
                    vps = vps_pool.tile([P, VC], F32, tag="v")
                    for ht in range(HT):
                        nc.tensor.matmul(
                            vps, hidT[:, ht, st * P:(st + 1) * P], wv[:, ht, :],
                            start=(ht == 0), stop=(ht == HT - 1),
                        )
                    nc.scalar.copy(v_nat[:, st, :], vps)

            # ---------- phase 2 pools (attention, shared with phase 3) -----
            # These outlive the hidT scope, so they go on the "right"
            # allocation stack to keep each stack's release order LIFO.
            qkT_pool = ctx.enter_context(
                tc.tile_pool(name="qkT", bufs=2, side="right"))
            st_ps_pool = ctx.enter_context(
                tc.tile_pool(name="stpsum", bufs=2, space="PSUM", side="right"))
            u_ps_pool = ctx.enter_context(
                tc.tile_pool(name="upsum", bufs=2, space="PSUM", side="right"))
            zb_ps_pool = ctx.enter_context(
                tc.tile_pool(name="zpsum", bufs=1, space="PSUM", side="right"))
            e_pool = ctx.enter_context(
                tc.tile_pool(name="epool", bufs=3, side="right"))
            es_pool = ctx.enter_context(
                tc.tile_pool(name="espool", bufs=1, side="right"))
            z_pool = ctx.enter_context(
                tc.tile_pool(name="zpool", bufs=2, side="right"))
            att_pool = ctx.enter_context(
                tc.tile_pool(name="attst", bufs=2, side="right"))

            def attention_piece(h, j):
                """Emit attention for head h, q-slice j (causal, transposed
                layout).  S^T tiles -> exp -> diag mask -> U^T + Z-broadcast
                PSUM accumulation (lag-1) -> 1/Z -> normalize."""
                qk = qkT_tiles[h]
                qsl = qk[:, 0, j * SL:(j + 1) * SL]
                nkt = 4 * j + 4
                u_ps = u_ps_pool.tile([P, SL], F32, tag="u")
                esum = es_pool.tile([P, SL], F32R, tag="es")
                ets = []

                def emit_u(i):
                    r = i - 4 * j
                    off = max(0, r) * P
                    nc.tensor.matmul(
                        u_ps[:, off:],
                        v_nat[:, i, h * D:(h + 1) * D],
                        ets[i][:, off:],
                        start=(i == 0), stop=(i == nkt - 1),
                    )

                for i in range(nkt):
                    r = i - 4 * j
                    off = max(0, r) * P
                    st_ps = st_ps_pool.tile([P, SL], F32, tag="st")
                    nc.tensor.matmul(
                        st_ps[:, off:],
                        qk[:, 1, i * P:(i + 1) * P],
                        qsl[:, off:],
                        start=True, stop=True,
                    )
                    et = e_pool.tile([P, SL], BF16, tag="e")
                    nc.scalar.activation(et[:, off:], st_ps[:, off:], EXPF,
                                         scale=NORM)
                    if r >= 0:
                        nc.vector.tensor_tensor(
                            et[:, off:off + P], et[:, off:off + P],
                            tri01, AluOpType.mult)
                    ets.append(et)
                    if i == 0:
                        nc.vector.tensor_copy(esum, et)
                    elif r < 1:
                        nc.vector.tensor_tensor(
                            esum, esum.bitcast(F32), et, AluOpType.add)
                    else:
                        nc.vector.tensor_tensor(
                            esum[:, off:], esum.bitcast(F32)[:, off:],
                            et[:, off:], AluOpType.add)
                    if i >= 1:
                        emit_u(i - 1)
                emit_u(nkt - 1)
                zb_ps = zb_ps_pool.tile([P, SL], F32, tag="zb")
                nc.tensor.matmul(zb_ps, ones_r, esum, start=True, stop=True)
                zr = z_pool.tile([P, SL], F32, tag="zr")
                nc.vector.reciprocal_approx_fast(zr, zb_ps)
                att = att_pool.tile([P, SL], BF16, tag="att")
                nc.vector.tensor_tensor(att, u_ps, zr, AluOpType.mult)
                nc.sync.dma_start(attn_loc[h, j], att)

            def gather_head(h):
                nc.gpsimd.collective_compute(
                    "AllGather", AluOpType.bypass,
                    replica_groups=[list(range(N_CORES))],
                    ins=[attn_loc[h].opt()],
                    outs=[attn_all[h][:].opt()],
                )

            # ---------- phase 2: per-head QK projection + RoPE ----------
            with ExitStack() as ab2:
                wq_pool = ab2.enter_context(tc.tile_pool(name="wq", bufs=1))
                wk_pool = ab2.enter_context(tc.tile_pool(name="wk", bufs=1))
                qkps_pool = ab2.enter_context(
                    tc.tile_pool(name="qkpsum", bufs=2, space="PSUM"))
                rstage = ab2.enter_context(tc.tile_pool(name="rstage", bufs=2))

                def dma_w(pool, part, h, tag):
                    wt = pool.tile([P, HT, D], BF16, tag=tag)
                    nc.sync.dma_start(
                        wt,
                        wqk_d[:, part * VC + h * D:part * VC + (h + 1) * D]
                        .rearrange("(t p) d -> p t d", p=P))
                    return wt

                w_tiles = {(0, 0): dma_w(wq_pool, 0, 0, "wq"),
                           (1, 0): dma_w(wk_pool, 1, 0, "wk")}

                for h in range(NH_LOC):
                    qkT = qkT_pool.tile([P, 2, S], BF16, tag="qkT")
                    qkT_tiles[h] = qkT
                    for part in range(2):  # 0 = q, 1 = k
                        wt = w_tiles.pop((part, h))
                        for sl in range(NSL):
                            qk_ps = qkps_pool.tile([P, SL], F32, tag="qk")
                            for ht in range(HT):
                                nc.tensor.matmul(
                                    qk_ps, wt[:, ht, :],
                                    hidT[:, ht, sl * SL:(sl + 1) * SL],
                                    start=(ht == 0), stop=(ht == HT - 1),
                                )
                            # RoPE: qkT[part, sl] = qk*cos + rot(qk)*sin_signed
                            # (rot = unsigned half-swap via partition-offset
                            # DMAs; the sign lives in the host sin table)
                            qt = rstage.tile([P, SL], BF16, tag="qt")
                            nc.scalar.copy(qt, qk_ps)
                            rot = rstage.tile([P, SL], BF16, tag="rot")
                            nc.sync.dma_start(rot[0:64, :], qt[64:128, :])
                            nc.sync.dma_start(rot[64:128, :], qt[0:64, :])
                            csl = cosT[:, sl * SL:(sl + 1) * SL]
                            ssl = sinT[:, sl * SL:(sl + 1) * SL]
                            nc.vector.tensor_tensor(qt, qt, csl, AluOpType.mult)
                            nc.vector.tensor_tensor(rot, rot, ssl, AluOpType.mult)
                            nc.vector.tensor_tensor(
                                qkT[:, part, sl * SL:(sl + 1) * SL], qt, rot,
                                AluOpType.add)
                            # interleave attention into the PE stream so its
                            # ACT/DVE work overlaps projection matmuls; the
                            # last two heads' pieces run as soon as their
                            # data is ready so the AllGather cascade clears
                            # before o_proj needs it.
                            if h in (1, 2) and part == 0:
                                attention_piece(h - 1, sl)
                                if sl == 3:
                                    gather_head(h - 1)
                            if h == 2 and part == 1 and sl == 3:
                                for j in range(NSL):
                                    attention_piece(h, j)
                                gather_head(h)
                            if h == NH_LOC - 1 and part == 1:
                                attention_piece(h, sl)
                                nc.gpsimd.collective_compute(
                                    "AllGather", AluOpType.bypass,
                                    replica_groups=[list(range(N_CORES))],
                                    ins=[attn_loc[h, sl].opt()],
                                    outs=[attn_all3[sl][:].opt()],
                                )
                        # prefetch next head's weights into the freed buffer
                        if h + 1 < NH_LOC:
                            pool = wq_pool if part == 0 else wk_pool
                            tag = "wq" if part == 0 else "wk"
                            w_tiles[(part, h + 1)] = dma_w(pool, part, h + 1, tag)

        # ---------- phase 3: head-3 attention + o_proj ----------
        wo_pool = ctx.enter_context(tc.tile_pool(name="wo", bufs=1))
        wo = wo_pool.tile([P, HT, OC], BF16)  # 32 KB/part
        for ot in range(4):  # split so o_proj's first group starts sooner
            nc.sync.dma_start(
                wo[:, :, ot * P:(ot + 1) * P],
                wo_d[:, ot * P:(ot + 1) * P].rearrange("(t p) o -> p t o", p=P))

        a_pool = ctx.enter_context(tc.tile_pool(name="apool", bufs=2))
        o_ps_pool = ctx.enter_context(
            tc.tile_pool(name="opsum", bufs=1, space="PSUM"))
        o_stage = ctx.enter_context(tc.tile_pool(name="ostage", bufs=2))

        hlast = NH_LOC - 1
        for sl in range(NSL):
            # heads 0..2 are gathered long before o_proj runs, so their
            # moving tiles prefetch freely; only head 3's small tile waits
            # on the per-slice gather
            af = a_pool.tile([P, (NH_LOC - 1) * N_CORES, SL], BF16, tag="af")
            for hh in range(NH_LOC - 1):
                nc.sync.dma_start(
                    af[:, hh * N_CORES:(hh + 1) * N_CORES, :],
                    attn_all[hh][:, sl].rearrange("c p s -> p c s"))
            af3 = a_pool.tile([P, N_CORES, SL], BF16, tag="af3")
            nc.sync.dma_start(
                af3, attn_all3[sl][:].rearrange("c p s -> p c s"))
            for og in range(2):
                ops = [o_ps_pool.tile([P, SL], F32, tag=f"o{t}",
                                      name=f"ops{t}")
                       for t in range(2)]
                for ci, (hh, cc) in enumerate(
                        (hh, cc) for hh in range(NH_LOC)
                        for cc in range(N_CORES)):
                    mv = (af[:, hh * N_CORES + cc, :] if hh < NH_LOC - 1
                          else af3[:, cc, :])
                    wcol = wo[:, cc * NH_LOC + hh, :]
                    for t in range(2):
                        ot = og * 2 + t
                        nc.tensor.matmul(
                            ops[t], wcol[:, ot * P:(ot + 1) * P], mv,
                            start=(ci == 0), stop=(ci == HT - 1),
                        )
                for t in range(2):
                    ot = og * 2 + t
                    ob = o_stage.tile([P, SL], BF16, tag="ob")
                    nc.scalar.copy(ob, ops[t])
                    nc.sync.dma_start(
                        out_d[ot * P:(ot + 1) * P, sl * SL:(sl + 1) * SL], ob)

    nc.compile()
    return nc


def make_in_maps(hidden_states, position_ids, W_pack, W_o):
    hidden = np.ascontiguousarray(
        np.asarray(hidden_states, dtype=np.float32).reshape(S, H))
    W_pack = np.asarray(W_pack, dtype=np.float32)
    W_o = np.asarray(W_o, dtype=np.float32)
    pos = np.asarray(position_ids).reshape(S).astype(np.float64)

    inv_freq = 1.0 / (10000.0 ** (np.arange(0, D, 2, dtype=np.float64) / D))
    freqs = np.outer(pos, inv_freq)  # [S, D/2]
    emb = np.concatenate([freqs, freqs], axis=1)  # [S, D]
    cos_t = np.ascontiguousarray(
        np.cos(emb).T.astype(ml_dtypes.bfloat16))  # [D, S]
    # sign-folded sin table: the device computes rot(q)[d] = q[(d+64)%128]
    # (unsigned swap), so rows 0..63 carry the rotate_half minus sign
    sin_full = np.sin(emb).T  # [D, S]
    sin_full[:D // 2] *= -1.0
    sin_t = np.ascontiguousarray(sin_full.astype(ml_dtypes.bfloat16))

    bf = ml_dtypes.bfloat16
    in_maps = []
    for c in range(N_CORES):
        qrows = W_pack[c * OC:(c + 1) * OC]
        krows = W_pack[H + c * OC:H + (c + 1) * OC]
        vrows = W_pack[2 * H + c * OC:2 * H + (c + 1) * OC]
        wqk_t = np.ascontiguousarray(
            np.concatenate([qrows, krows], axis=0).T.astype(bf))  # [H, 1024]
        wv_t = np.ascontiguousarray(vrows.T.astype(bf))  # [H, 512]
        wo_t = np.ascontiguousarray(W_o[c * OC:(c + 1) * OC].T.astype(bf))
        in_maps.append({
            "hidden": hidden,
            "wqk_t": wqk_t,
            "wv_t": wv_t,
            "wo_t": wo_t,
            "cos_t": cos_t,
            "sin_t": sin_t,
        })
    return in_maps


_NC_CACHE = None


def get_nc():
    global _NC_CACHE
    if _NC_CACHE is None:
        _NC_CACHE = build_nc()
    return _NC_CACHE


def run(inputs, trace=False):
    """Run on hardware; returns (output [1,S,H] f32, BassKernelResults)."""
    in_maps = make_in_maps(
        inputs["hidden_states"], inputs["position_ids"],
        inputs["W_pack"], inputs["W_o"])
    nc = get_nc()
    res = run_bass_kernel_spmd(nc, in_maps, list(range(N_CORES)), trace=trace)
    parts = [np.asarray(res.results[c]["out_t"]).astype(np.float32)
             for c in range(N_CORES)]
    out_t = np.concatenate(parts, axis=0)  # [H, S]
    out = np.ascontiguousarray(out_t.T)[None]  # [1, S, H]
    return out.astype(np.float32), res


def kernel(**inputs):
    out, _ = run(inputs, trace=False)
    return out
